# revision 37
# baseline (speedup 1.0000x reference)
"""Causal self-attention (B=4, S=2048, E=1024, D=128, single head) on 8 TRN2 cores.

Sharding: core c = 2*b + h handles batch b; the two cores of a pair split the
causal key range by k-tile parity (h=0 even 128-row k-tiles, h=1 odd). All 8
cores run the *same* instruction stream; per-core differences live in DRAM
data only:
  - xt [1024, 2048] fp16: x[b].T with 128-col s-tiles stored in "perm order"
    (position p holds global tile p^h), so EVEN positions are always the
    core's own-parity tiles. K/V projection reads even positions via a
    strided AP; Q projection reads all positions in storage order.
  - mask [128, 256] fp16: additive score mask for the two diagonal-region
    k-tiles of every q-block ([tri | 0] for h=0, [tri | -6e4] for h=1).

Attention runs over four contiguous 512-col q-blocks (perm order ~ natural
order up to intra-pair swaps). Block J attends local k-tiles i=0..2J+1 with
an exact-causal trapezoid: tile i=2J+1 streams only columns [256:512); tiles
i=2J and 2J+1 get the additive mask (DVE, off the PE) before exp. Softmax
denominators come from accumulating exp tiles into ACC (DVE fp16 2x-rate
adds) and ONE rank-1 ones^T @ ACC matmul per block, instead of one per
k-tile. PSUM->SBUF output staging copies run on the ACT engine (DVE was the
busier engine). All PE operands are fp16; PSUM stays fp32. Per-core PE
streaming ~54k nominal cycles vs ~79k for the f32r/mask-matmul/per-tile-sums
baseline - and fp16 moving operands stream ~2 cols/cycle on HW, which f32r
does not, so the PE-bound steady state roughly halves on top of that.

Each core emits unnormalized PV partials (pvt [128 d, 2048 q perm]) and
denominators (sums [1, 2048]); the host un-permutes and combines the pair:
  out[b] = ((pv0 + pv1) / (s0 + s1)).T

Measured (reps-delta wall-clock, see prof.py): rel err 5.4e-4; ~12 us/iter
steady state (median-of-rounds 12.2, fast rounds 9.3-11.4) vs ~30.6 us for
the staged baseline under the same methodology (grader-reported baseline:
26.6 us). Compute-bound: a probe build that skips the 4 MB xt upload still
takes ~12.4 us/iter, i.e. PE streaming is ~90% of the critical path. A
phase-controlled A/B showed sequential per-chunk emission beats a
software-pipelined (attention-one-group-behind) emission in every round;
the in-block masked-tiles-early order is what the sequential path keeps.
"""

import os

os.environ.setdefault("MYCRO_LOCAL_CACHE", "1")

import numpy as np

B, S, E, D = 4, 2048, 1024, 128
P = 128
NT = S // P          # 16 global s-tiles per batch
LT = NT // 2         # 8 local (per-core) k-tiles
NQB = 4              # 512-wide query blocks
QBW = 512
NEB = E // P         # 8 e-tiles
SCALE = 1.0 / float(np.sqrt(D))
NEG = -60000.0       # fp16-representable; exp underflows to 0 in fp32

TRACE = False        # set by test.py for profiling runs
TRACE_KW = {}
PROBE_NO_XT = False  # timing probe: skip xt DMA (results wrong; perf only)

_CACHE = {}


def _build_module(reps=1, pipeline=False):
    from contextlib import ExitStack

    import concourse.bacc as bacc
    import concourse.mybir as mybir
    import concourse.tile as tile

    f32 = mybir.dt.float32
    f16 = mybir.dt.float16

    nc = bacc.Bacc("TRN2", target_bir_lowering=False, debug=False, num_devices=8)

    xt_d = nc.dram_tensor("xt", [E, S], f16, kind="ExternalInput").ap()
    wq_d = nc.dram_tensor("wq", [E, D], f16, kind="ExternalInput").ap()
    wk_d = nc.dram_tensor("wk", [E, D], f16, kind="ExternalInput").ap()
    wv_d = nc.dram_tensor("wv", [E, D], f16, kind="ExternalInput").ap()
    bq_d = nc.dram_tensor("bq", [D], f32, kind="ExternalInput").ap()  # pre-scaled
    bk_d = nc.dram_tensor("bk", [D], f32, kind="ExternalInput").ap()
    bv_d = nc.dram_tensor("bv", [D], f32, kind="ExternalInput").ap()
    mask_d = nc.dram_tensor("mask", [P, 2 * P], f16, kind="ExternalInput").ap()
    ones_d = nc.dram_tensor("ones", [P, 1], f16, kind="ExternalInput").ap()
    ident_d = nc.dram_tensor("ident", [P, P], f16, kind="ExternalInput").ap()
    pvt_d = nc.dram_tensor("pvt", [D, S], f32, kind="ExternalOutput").ap()
    sums_d = nc.dram_tensor("sums", [1, S], f32, kind="ExternalOutput").ap()

    with tile.TileContext(nc) as tc, ExitStack() as ctx:
        singles = ctx.enter_context(tc.tile_pool(name="singles", bufs=1))
        ppool = ctx.enter_context(tc.tile_pool(name="ppool", bufs=4))
        apool = ctx.enter_context(tc.tile_pool(name="apool", bufs=2))
        proj_ps = ctx.enter_context(tc.tile_pool(name="proj_ps", bufs=1, space="PSUM"))
        tr_ps = proj_ps
        sc_ps = ctx.enter_context(tc.tile_pool(name="sc_ps", bufs=3, space="PSUM"))
        pv_ps = ctx.enter_context(tc.tile_pool(name="pv_ps", bufs=1, space="PSUM"))
        sum_ps = ctx.enter_context(tc.tile_pool(name="sum_ps", bufs=1, space="PSUM"))

        # ---- constants (ACT HWDGE ring; xt stream owns the SP ring) ----
        w_sb = {}
        for name, dram in (("wk", wk_d), ("wv", wv_d), ("wq", wq_d)):
            t = singles.tile([P, NEB, D], f16, tag=f"w_{name}")
            nc.scalar.dma_start(t[:], dram.rearrange("(o p) d -> p o d", p=P))
            w_sb[name] = t
        b_sb = {}
        for name, dram in (("bq", bq_d), ("bk", bk_d), ("bv", bv_d)):
            t = singles.tile([P, 1], f32, tag=f"b_{name}")
            nc.scalar.dma_start(t[:], dram.rearrange("(p one) -> p one", one=1))
            b_sb[name] = t
        mask_sb = singles.tile([P, 2 * P], f16, tag="mask")
        nc.scalar.dma_start(mask_sb[:], mask_d[:])
        ones = singles.tile([P, 1], f16, tag="ones")
        nc.scalar.dma_start(ones[:], ones_d[:])
        ident = singles.tile([P, P], f16, tag="ident")
        nc.scalar.dma_start(ident[:], ident_d[:])

        # ---- persistent activations ----
        # xt resident in perm order: [e-part, e-tile, pair, parity-pos, col]
        xt_sb = singles.tile([P, NEB, LT, 2, P], f16, tag="xt_sb")
        kt = singles.tile([P, LT, P], f16, tag="kt")      # K^T  [d, lt, k]
        vt = singles.tile([P, LT, P], f16, tag="vt")      # V^T  [d, lt, k]
        vn = singles.tile([P, LT, D], f16, tag="vn")      # V natural [k, lt, d]
        qt = singles.tile([P, NT, P], f16, tag="qt")      # Q^T [d, pos, q]
        pvt_sb = singles.tile([D, S], f32, tag="pvt_sb")
        sums_sb = singles.tile([1, S], f32, tag="sums_sb")

        ktv = kt.rearrange("p lt k -> p (lt k)")
        vtv = vt.rearrange("p lt k -> p (lt k)")

        def load_chunk(J, rep=0):
            """DMA xt columns [512J, 512J+512) (positions 4J..4J+3)."""
            if PROBE_NO_XT and rep > 0:
                return
            for eo in range(NEB):
                eng = nc.sync if eo % 2 == 0 else nc.scalar
                eng.dma_start(
                    xt_sb[:, eo, 2 * J : 2 * J + 2, :, :],
                    xt_d[eo * P : (eo + 1) * P, J * QBW : (J + 1) * QBW],
                )

        def proj_chunk(J):
            """K/V for local k-tiles {2J, 2J+1} + Q for block J."""
            for name, dstv, bias in (("wk", ktv, "bk"), ("wv", vtv, "bv")):
                ps = proj_ps.tile([P, QBW], f32, tag="ps_kv")
                for eo in range(NEB):
                    nc.tensor.matmul(
                        ps[:, : 2 * P],
                        w_sb[name][:, eo, :],
                        xt_sb[:, eo, 2 * J : 2 * J + 2, 0, :],
                        start=(eo == 0),
                        stop=(eo == NEB - 1),
                    )
                nc.vector.tensor_scalar_add(
                    dstv[:, J * 2 * P : (J + 1) * 2 * P], ps[:, : 2 * P], b_sb[bias][:]
                )
            ps = proj_ps.tile([P, QBW], f32, tag="ps_q")
            for eo in range(NEB):
                nc.tensor.matmul(
                    ps[:],
                    w_sb["wq"][:, eo, :],
                    xt_sb[:, eo, 2 * J : 2 * J + 2, :, :],
                    start=(eo == 0),
                    stop=(eo == NEB - 1),
                )
            qv = qt.rearrange("p t q -> p (t q)")
            nc.vector.tensor_scalar(
                qv[:, J * QBW : (J + 1) * QBW],
                ps[:],
                SCALE,
                b_sb["bq"][:],
                mybir.AluOpType.mult,
                mybir.AluOpType.add,
            )

        def v_transpose(lt):
            ps = tr_ps.tile([P, P], f16, tag="tr")
            nc.tensor.transpose(ps[:], vt[:, lt, :], ident[:])
            nc.vector.tensor_copy(out=vn[:, lt, :], in_=ps[:])

        def attention_blk(J):
            """Block J: q-cols [512J, 512J+512), local k-tiles 0..2J+1."""
            nlt = 2 * J + 2
            col0 = J * QBW
            pv = pv_ps.tile([P, QBW], f32, tag="pv")
            acc = apool.tile([P, QBW], f16, tag="acc")
            # emission order: full-width i=0 first (clears PSUM over the
            # whole block), then the masked tiles (their DVE-mask + exp
            # latency hides behind the remaining scores matmuls), ending on
            # a plain tile so only one exp latency is exposed at block end.
            if J == 0:
                order = [0, 1]
            else:
                order = [0, 2 * J, 2 * J + 1] + list(range(1, 2 * J))
            for idx, i in enumerate(order):
                c0 = 2 * P if i == 2 * J + 1 else 0
                sc = sc_ps.tile([P, QBW], f32, tag="sc")
                nc.tensor.matmul(
                    sc[:, c0:],
                    kt[:, i, :],
                    qt[:, 4 * J + c0 // P : 4 * J + 4, :],
                    start=True,
                    stop=True,
                )
                if i >= 2 * J:
                    nc.vector.tensor_tensor(
                        out=sc[:, c0 : c0 + 2 * P],
                        in0=sc[:, c0 : c0 + 2 * P],
                        in1=mask_sb[:],
                        op=mybir.AluOpType.add,
                    )
                if idx == 0:
                    psrc = acc
                    nc.scalar.activation(
                        acc[:], sc[:], mybir.ActivationFunctionType.Exp
                    )
                else:
                    psrc = ppool.tile([P, QBW], f16, tag="p")
                    nc.scalar.activation(
                        psrc[:, c0:], sc[:, c0:], mybir.ActivationFunctionType.Exp
                    )
                    nc.vector.tensor_tensor(
                        out=acc[:, c0:],
                        in0=acc[:, c0:],
                        in1=psrc[:, c0:],
                        op=mybir.AluOpType.add,
                    )
                nc.tensor.matmul(
                    pv[:, c0:],
                    vn[:, i, :],
                    psrc[:, c0:],
                    start=(idx == 0),
                    stop=(idx == nlt - 1),
                    skip_group_check=True,
                )
            return pv, acc

        def finish_blk(J, pv, acc):
            """Block J tail: denominator matmul, staging copies, output DMA.
            Emitted one chunk-group after attention_blk(J) so the PE has
            projection work queued while ACT/DVE drain the block tail."""
            col0 = J * QBW
            sm = sum_ps.tile([1, QBW], f32, tag="sm")
            nc.tensor.matmul(sm[:], ones[:], acc[:], start=True, stop=True)
            nc.scalar.copy(out=pvt_sb[:, col0 : col0 + QBW], in_=pv[:])
            nc.scalar.copy(out=sums_sb[:, col0 : col0 + QBW], in_=sm[:])
            out_eng = nc.gpsimd
            out_eng.dma_start(
                pvt_d[:, col0 : col0 + QBW], pvt_sb[:, col0 : col0 + QBW]
            )
            out_eng.dma_start(
                sums_d[:, col0 : col0 + QBW], sums_sb[:, col0 : col0 + QBW]
            )

        # ---- emission order (priority hint for the scheduler) ----
        # Software pipeline: group k emits chunk-k%4's DMA + projections,
        # then finish(k-2), then attention(k-1). Attention never sits at the
        # head of the PE queue behind its own block tail, and each block's
        # exposed exp/acc latency is covered by the next chunk's projections
        # (across the rep boundary too).
        pend_attn = None   # J of attention not yet emitted
        pend_fin = None    # (J, pv, acc) of finish not yet emitted
        for _rep in range(reps):
            for J in range(NQB):
                load_chunk(J, _rep)
                proj_chunk(J)
                v_transpose(2 * J)
                v_transpose(2 * J + 1)
                if not pipeline:
                    finish_blk(J, *attention_blk(J))
                    continue
                if pend_fin is not None:
                    finish_blk(*pend_fin)
                    pend_fin = None
                if pend_attn is not None:
                    pend_fin = (pend_attn, *attention_blk(pend_attn))
                pend_attn = J
        # drain the pipeline tail
        if pend_fin is not None:
            finish_blk(*pend_fin)
        if pend_attn is not None:
            finish_blk(pend_attn, *attention_blk(pend_attn))

    nc.compile()
    return nc


def _get_module(reps=1, pipeline=False):
    key = ("nc", reps, pipeline)
    if key not in _CACHE:
        _CACHE[key] = _build_module(reps, pipeline)
    return _CACHE[key]


def _host_prep(x, Wq, bq, Wk, bk, Wv, bv):
    """Build the 8 per-core input maps plus per-core q-column permutations."""
    x = np.asarray(x, dtype=np.float32)
    tri = np.where(
        np.arange(P)[None, :] >= np.arange(P)[:, None], 0.0, NEG
    ).astype(np.float16)
    in_maps = []
    perms = []
    for c in range(8):
        b, h = divmod(c, 2)
        xt3 = np.ascontiguousarray(x[b].T).reshape(E, NT, P)
        # perm order: position p holds global tile p^h
        pos = np.arange(NT) ^ h
        xt_perm = np.ascontiguousarray(xt3[:, pos, :].reshape(E, S)).astype(
            np.float16
        )
        mask = np.concatenate(
            [tri, np.full((P, P), 0.0 if h == 0 else NEG, np.float16)], axis=1
        )
        in_maps.append(
            {
                "xt": xt_perm,
                "wq": np.asarray(Wq, np.float16),
                "wk": np.asarray(Wk, np.float16),
                "wv": np.asarray(Wv, np.float16),
                "bq": np.asarray(bq, np.float32) * np.float32(SCALE),
                "bk": np.asarray(bk, np.float32),
                "bv": np.asarray(bv, np.float32),
                "mask": np.ascontiguousarray(mask),
                "ones": np.ones((P, 1), dtype=np.float16),
                "ident": np.eye(P, dtype=np.float16),
            }
        )
        # storage col -> global q row (position tile p holds global tile p^h)
        perm = np.empty(S, dtype=np.int64)
        for t in range(NT):
            perm[t * P : (t + 1) * P] = (t ^ h) * P + np.arange(P)
        perms.append(perm)
    return in_maps, perms


def kernel(x, Wq, bq, Wk, bk, Wv, bv):
    from concourse.bass_utils import run_bass_kernel_spmd

    nc = _get_module()
    in_maps, perms = _host_prep(x, Wq, bq, Wk, bk, Wv, bv)
    res = run_bass_kernel_spmd(
        nc,
        in_maps,
        core_ids=list(range(8)),
        trace=TRACE,
        **TRACE_KW,
    )
    _CACHE["last_result"] = res

    out = np.empty((B, S, D), dtype=np.float32)
    for b in range(B):
        r0, r1 = res.results[2 * b], res.results[2 * b + 1]
        pv = np.zeros((D, S), dtype=np.float64)
        sm = np.zeros((S,), dtype=np.float64)
        for r, perm in ((r0, perms[2 * b]), (r1, perms[2 * b + 1])):
            pv[:, perm] += r["pvt"].astype(np.float64)
            sm[perm] += r["sums"][0].astype(np.float64)
        out[b] = (pv / sm[None, :]).T.astype(np.float32)
    return out


# revision 39
# speedup vs baseline: 1.2216x; 1.2216x over previous
"""Causal self-attention (B=4, S=2048, E=1024, D=128, single head) on 8 TRN2 cores.

Sharding: core c = 2*b + h handles batch b; the two cores of a pair split the
causal key range by k-tile parity (h=0 even 128-row k-tiles, h=1 odd). All 8
cores run the *same* instruction stream; per-core differences live in DRAM
data only:
  - xt [1024, 2048] fp16: x[b].T with 128-col s-tiles stored in "perm order"
    (position p holds global tile p^h), so EVEN positions are always the
    core's own-parity tiles. K/V projection reads even positions via a
    strided AP; Q projection reads all positions in storage order.
  - mask [128, 256] fp16: additive score mask for the two diagonal-region
    k-tiles of every q-block ([tri | 0] for h=0, [tri | -6e4] for h=1).

Attention runs over four contiguous 512-col q-blocks (perm order ~ natural
order up to intra-pair swaps). Block J attends local k-tiles i=0..2J+1 with
an exact-causal trapezoid: tile i=2J+1 streams only columns [256:512); tiles
i=2J and 2J+1 get the additive mask (DVE, off the PE) before exp. Softmax
denominators come from accumulating exp tiles into ACC (DVE fp16 2x-rate
adds) and ONE rank-1 ones^T @ ACC matmul per block, instead of one per
k-tile. PSUM->SBUF output staging copies run on the ACT engine (DVE was the
busier engine). All PE operands are fp16; PSUM stays fp32. Per-core PE
streaming ~54k nominal cycles vs ~79k for the f32r/mask-matmul/per-tile-sums
baseline - and fp16 moving operands stream ~2 cols/cycle on HW, which f32r
does not, so the PE-bound steady state roughly halves on top of that.

Each core emits unnormalized PV partials (pvt [128 d, 2048 q perm]) and
denominators (sums [1, 2048]); the host un-permutes and combines the pair:
  out[b] = ((pv0 + pv1) / (s0 + s1)).T

Measured (reps-delta wall-clock, see prof.py): rel err 5.4e-4; ~12 us/iter
steady state (median-of-rounds 12.2, fast rounds 9.3-11.4) vs ~30.6 us for
the staged baseline under the same methodology (grader-reported baseline:
26.6 us). Compute-bound: a probe build that skips the 4 MB xt upload still
takes ~12.4 us/iter, i.e. PE streaming is ~90% of the critical path. A
phase-controlled A/B showed sequential per-chunk emission beats a
software-pipelined (attention-one-group-behind) emission in every round;
the in-block masked-tiles-early order is what the sequential path keeps.
"""

import os

os.environ.setdefault("MYCRO_LOCAL_CACHE", "1")

import numpy as np

B, S, E, D = 4, 2048, 1024, 128
P = 128
NT = S // P          # 16 global s-tiles per batch
LT = NT // 2         # 8 local (per-core) k-tiles
NQB = 4              # 512-wide query blocks
QBW = 512
NEB = E // P         # 8 e-tiles
SCALE = 1.0 / float(np.sqrt(D))
NEG = -60000.0       # fp16-representable; exp underflows to 0 in fp32

TRACE = False        # set by test.py for profiling runs
TRACE_KW = {}
PROBE_NO_XT = False  # timing probe: skip xt DMA (results wrong; perf only)

_CACHE = {}


def _build_module(reps=1, pipeline=False, split_rings=False):
    from contextlib import ExitStack

    import concourse.bacc as bacc
    import concourse.mybir as mybir
    import concourse.tile as tile

    f32 = mybir.dt.float32
    f16 = mybir.dt.float16

    nc = bacc.Bacc("TRN2", target_bir_lowering=False, debug=False, num_devices=8)

    xt_d = nc.dram_tensor("xt", [E, S], f16, kind="ExternalInput").ap()
    wq_d = nc.dram_tensor("wq", [E, D], f16, kind="ExternalInput").ap()
    wk_d = nc.dram_tensor("wk", [E, D], f16, kind="ExternalInput").ap()
    wv_d = nc.dram_tensor("wv", [E, D], f16, kind="ExternalInput").ap()
    bq_d = nc.dram_tensor("bq", [D], f32, kind="ExternalInput").ap()  # pre-scaled
    bk_d = nc.dram_tensor("bk", [D], f32, kind="ExternalInput").ap()
    bv_d = nc.dram_tensor("bv", [D], f32, kind="ExternalInput").ap()
    mask_d = nc.dram_tensor("mask", [P, 2 * P], f16, kind="ExternalInput").ap()
    ones_d = nc.dram_tensor("ones", [P, 1], f16, kind="ExternalInput").ap()
    ident_d = nc.dram_tensor("ident", [P, P], f16, kind="ExternalInput").ap()
    pvt_d = nc.dram_tensor("pvt", [D, S], f32, kind="ExternalOutput").ap()
    sums_d = nc.dram_tensor("sums", [1, S], f32, kind="ExternalOutput").ap()

    with tile.TileContext(nc) as tc, ExitStack() as ctx:
        singles = ctx.enter_context(tc.tile_pool(name="singles", bufs=1))
        ppool = ctx.enter_context(tc.tile_pool(name="ppool", bufs=4))
        apool = ctx.enter_context(tc.tile_pool(name="apool", bufs=2))
        proj_ps = ctx.enter_context(tc.tile_pool(name="proj_ps", bufs=1, space="PSUM"))
        tr_ps = proj_ps
        sc_ps = ctx.enter_context(tc.tile_pool(name="sc_ps", bufs=3, space="PSUM"))
        pv_ps = ctx.enter_context(tc.tile_pool(name="pv_ps", bufs=1, space="PSUM"))
        sum_ps = ctx.enter_context(tc.tile_pool(name="sum_ps", bufs=1, space="PSUM"))

        # ---- constants (ACT HWDGE ring; xt stream owns the SP ring) ----
        w_sb = {}
        for name, dram in (("wk", wk_d), ("wv", wv_d), ("wq", wq_d)):
            t = singles.tile([P, NEB, D], f16, tag=f"w_{name}")
            nc.scalar.dma_start(t[:], dram.rearrange("(o p) d -> p o d", p=P))
            w_sb[name] = t
        b_sb = {}
        for name, dram in (("bq", bq_d), ("bk", bk_d), ("bv", bv_d)):
            t = singles.tile([P, 1], f32, tag=f"b_{name}")
            nc.scalar.dma_start(t[:], dram.rearrange("(p one) -> p one", one=1))
            b_sb[name] = t
        mask_sb = singles.tile([P, 2 * P], f16, tag="mask")
        nc.scalar.dma_start(mask_sb[:], mask_d[:])
        ones = singles.tile([P, 1], f16, tag="ones")
        nc.scalar.dma_start(ones[:], ones_d[:])
        ident = singles.tile([P, P], f16, tag="ident")
        nc.scalar.dma_start(ident[:], ident_d[:])

        # ---- persistent activations ----
        # xt resident in perm order: [e-part, e-tile, pair, parity-pos, col]
        xt_sb = singles.tile([P, NEB, LT, 2, P], f16, tag="xt_sb")
        kt = singles.tile([P, LT, P], f16, tag="kt")      # K^T  [d, lt, k]
        vt = singles.tile([P, LT, P], f16, tag="vt")      # V^T  [d, lt, k]
        vn = singles.tile([P, LT, D], f16, tag="vn")      # V natural [k, lt, d]
        qt = singles.tile([P, NT, P], f16, tag="qt")      # Q^T [d, pos, q]
        pvt_sb = singles.tile([D, S], f32, tag="pvt_sb")
        sums_sb = singles.tile([1, S], f32, tag="sums_sb")

        ktv = kt.rearrange("p lt k -> p (lt k)")
        vtv = vt.rearrange("p lt k -> p (lt k)")

        def load_chunk(J, rep=0):
            """DMA xt columns [512J, 512J+512) (positions 4J..4J+3)."""
            if PROBE_NO_XT and rep > 0:
                return
            for eo in range(NEB):
                eng = nc.sync if (eo % 2 == 0 or not split_rings) else nc.scalar
                eng.dma_start(
                    xt_sb[:, eo, 2 * J : 2 * J + 2, :, :],
                    xt_d[eo * P : (eo + 1) * P, J * QBW : (J + 1) * QBW],
                )

        def proj_chunk(J):
            """K/V for local k-tiles {2J, 2J+1} + Q for block J."""
            for name, dstv, bias in (("wk", ktv, "bk"), ("wv", vtv, "bv")):
                ps = proj_ps.tile([P, QBW], f32, tag="ps_kv")
                for eo in range(NEB):
                    nc.tensor.matmul(
                        ps[:, : 2 * P],
                        w_sb[name][:, eo, :],
                        xt_sb[:, eo, 2 * J : 2 * J + 2, 0, :],
                        start=(eo == 0),
                        stop=(eo == NEB - 1),
                    )
                nc.vector.tensor_scalar_add(
                    dstv[:, J * 2 * P : (J + 1) * 2 * P], ps[:, : 2 * P], b_sb[bias][:]
                )
            ps = proj_ps.tile([P, QBW], f32, tag="ps_q")
            for eo in range(NEB):
                nc.tensor.matmul(
                    ps[:],
                    w_sb["wq"][:, eo, :],
                    xt_sb[:, eo, 2 * J : 2 * J + 2, :, :],
                    start=(eo == 0),
                    stop=(eo == NEB - 1),
                )
            qv = qt.rearrange("p t q -> p (t q)")
            nc.vector.tensor_scalar(
                qv[:, J * QBW : (J + 1) * QBW],
                ps[:],
                SCALE,
                b_sb["bq"][:],
                mybir.AluOpType.mult,
                mybir.AluOpType.add,
            )

        def v_transpose(lt):
            ps = tr_ps.tile([P, P], f16, tag="tr")
            nc.tensor.transpose(ps[:], vt[:, lt, :], ident[:])
            nc.vector.tensor_copy(out=vn[:, lt, :], in_=ps[:])

        def attention_blk(J):
            """Block J: q-cols [512J, 512J+512), local k-tiles 0..2J+1."""
            nlt = 2 * J + 2
            col0 = J * QBW
            pv = pv_ps.tile([P, QBW], f32, tag="pv")
            acc = apool.tile([P, QBW], f16, tag="acc")
            # emission order: full-width i=0 first (clears PSUM over the
            # whole block), then the masked tiles (their DVE-mask + exp
            # latency hides behind the remaining scores matmuls), ending on
            # a plain tile so only one exp latency is exposed at block end.
            if J == 0:
                order = [0, 1]
            else:
                order = [0, 2 * J, 2 * J + 1] + list(range(1, 2 * J))
            for idx, i in enumerate(order):
                c0 = 2 * P if i == 2 * J + 1 else 0
                sc = sc_ps.tile([P, QBW], f32, tag="sc")
                nc.tensor.matmul(
                    sc[:, c0:],
                    kt[:, i, :],
                    qt[:, 4 * J + c0 // P : 4 * J + 4, :],
                    start=True,
                    stop=True,
                )
                if i >= 2 * J:
                    nc.vector.tensor_tensor(
                        out=sc[:, c0 : c0 + 2 * P],
                        in0=sc[:, c0 : c0 + 2 * P],
                        in1=mask_sb[:],
                        op=mybir.AluOpType.add,
                    )
                if idx == 0:
                    psrc = acc
                    nc.scalar.activation(
                        acc[:], sc[:], mybir.ActivationFunctionType.Exp
                    )
                else:
                    psrc = ppool.tile([P, QBW], f16, tag="p")
                    nc.scalar.activation(
                        psrc[:, c0:], sc[:, c0:], mybir.ActivationFunctionType.Exp
                    )
                    nc.vector.tensor_tensor(
                        out=acc[:, c0:],
                        in0=acc[:, c0:],
                        in1=psrc[:, c0:],
                        op=mybir.AluOpType.add,
                    )
                nc.tensor.matmul(
                    pv[:, c0:],
                    vn[:, i, :],
                    psrc[:, c0:],
                    start=(idx == 0),
                    stop=(idx == nlt - 1),
                    skip_group_check=True,
                )
            return pv, acc

        def finish_blk(J, pv, acc):
            """Block J tail: denominator matmul, staging copies, output DMA.
            Emitted one chunk-group after attention_blk(J) so the PE has
            projection work queued while ACT/DVE drain the block tail."""
            col0 = J * QBW
            sm = sum_ps.tile([1, QBW], f32, tag="sm")
            nc.tensor.matmul(sm[:], ones[:], acc[:], start=True, stop=True)
            nc.scalar.copy(out=pvt_sb[:, col0 : col0 + QBW], in_=pv[:])
            nc.scalar.copy(out=sums_sb[:, col0 : col0 + QBW], in_=sm[:])
            out_eng = nc.gpsimd
            out_eng.dma_start(
                pvt_d[:, col0 : col0 + QBW], pvt_sb[:, col0 : col0 + QBW]
            )
            out_eng.dma_start(
                sums_d[:, col0 : col0 + QBW], sums_sb[:, col0 : col0 + QBW]
            )

        # ---- emission order (priority hint for the scheduler) ----
        # Software pipeline: group k emits chunk-k%4's DMA + projections,
        # then finish(k-2), then attention(k-1). Attention never sits at the
        # head of the PE queue behind its own block tail, and each block's
        # exposed exp/acc latency is covered by the next chunk's projections
        # (across the rep boundary too).
        pend_attn = None   # J of attention not yet emitted
        pend_fin = None    # (J, pv, acc) of finish not yet emitted
        for _rep in range(reps):
            for J in range(NQB):
                load_chunk(J, _rep)
                proj_chunk(J)
                v_transpose(2 * J)
                v_transpose(2 * J + 1)
                if not pipeline:
                    finish_blk(J, *attention_blk(J))
                    continue
                if pend_fin is not None:
                    finish_blk(*pend_fin)
                    pend_fin = None
                if pend_attn is not None:
                    pend_fin = (pend_attn, *attention_blk(pend_attn))
                pend_attn = J
        # drain the pipeline tail
        if pend_fin is not None:
            finish_blk(*pend_fin)
        if pend_attn is not None:
            finish_blk(pend_attn, *attention_blk(pend_attn))

    nc.compile()
    return nc


def _get_module(reps=1, pipeline=False, split_rings=False):
    key = ("nc", reps, pipeline, split_rings)
    if key not in _CACHE:
        _CACHE[key] = _build_module(reps, pipeline, split_rings)
    return _CACHE[key]


def _host_prep(x, Wq, bq, Wk, bk, Wv, bv):
    """Build the 8 per-core input maps plus per-core q-column permutations."""
    x = np.asarray(x, dtype=np.float32)
    tri = np.where(
        np.arange(P)[None, :] >= np.arange(P)[:, None], 0.0, NEG
    ).astype(np.float16)
    in_maps = []
    perms = []
    for c in range(8):
        b, h = divmod(c, 2)
        xt3 = np.ascontiguousarray(x[b].T).reshape(E, NT, P)
        # perm order: position p holds global tile p^h
        pos = np.arange(NT) ^ h
        xt_perm = np.ascontiguousarray(xt3[:, pos, :].reshape(E, S)).astype(
            np.float16
        )
        mask = np.concatenate(
            [tri, np.full((P, P), 0.0 if h == 0 else NEG, np.float16)], axis=1
        )
        in_maps.append(
            {
                "xt": xt_perm,
                "wq": np.asarray(Wq, np.float16),
                "wk": np.asarray(Wk, np.float16),
                "wv": np.asarray(Wv, np.float16),
                "bq": np.asarray(bq, np.float32) * np.float32(SCALE),
                "bk": np.asarray(bk, np.float32),
                "bv": np.asarray(bv, np.float32),
                "mask": np.ascontiguousarray(mask),
                "ones": np.ones((P, 1), dtype=np.float16),
                "ident": np.eye(P, dtype=np.float16),
            }
        )
        # storage col -> global q row (position tile p holds global tile p^h)
        perm = np.empty(S, dtype=np.int64)
        for t in range(NT):
            perm[t * P : (t + 1) * P] = (t ^ h) * P + np.arange(P)
        perms.append(perm)
    return in_maps, perms


def kernel(x, Wq, bq, Wk, bk, Wv, bv):
    from concourse.bass_utils import run_bass_kernel_spmd

    nc = _get_module()
    in_maps, perms = _host_prep(x, Wq, bq, Wk, bk, Wv, bv)
    res = run_bass_kernel_spmd(
        nc,
        in_maps,
        core_ids=list(range(8)),
        trace=TRACE,
        **TRACE_KW,
    )
    _CACHE["last_result"] = res

    out = np.empty((B, S, D), dtype=np.float32)
    for b in range(B):
        r0, r1 = res.results[2 * b], res.results[2 * b + 1]
        pv = np.zeros((D, S), dtype=np.float64)
        sm = np.zeros((S,), dtype=np.float64)
        for r, perm in ((r0, perms[2 * b]), (r1, perms[2 * b + 1])):
            pv[:, perm] += r["pvt"].astype(np.float64)
            sm[perm] += r["sums"][0].astype(np.float64)
        out[b] = (pv / sm[None, :]).T.astype(np.float32)
    return out


# revision 44
# speedup vs baseline: 1.3371x; 1.0945x over previous
"""Causal self-attention (B=4, S=2048, E=1024, D=128, single head) on 8 TRN2 cores.

Sharding: core c = 2*b + h handles batch b; the two cores of a pair split the
causal key range by k-tile parity (h=0 even 128-row k-tiles, h=1 odd). All 8
cores run the *same* instruction stream; per-core differences live in DRAM
data only:
  - xt [1024, 2048] fp16: x[b].T with 128-col s-tiles stored in "perm order"
    (position p holds global tile p^h), so EVEN positions are always the
    core's own-parity tiles. K/V projection reads even positions via a
    strided AP; Q projection reads all positions in storage order.
  - mask [128, 256] fp16: additive score mask for the two diagonal-region
    k-tiles of every q-block ([tri | 0] for h=0, [tri | -6e4] for h=1).

Attention runs over four contiguous 512-col q-blocks (perm order ~ natural
order up to intra-pair swaps). Block J attends local k-tiles i=0..2J+1 with
an exact-causal trapezoid: tile i=2J+1 streams only columns [256:512); tiles
i=2J and 2J+1 get the additive mask (DVE, off the PE) before exp. Softmax
denominators come from accumulating exp tiles into ACC (DVE fp16 2x-rate
adds) and ONE rank-1 ones^T @ ACC matmul per block, instead of one per
k-tile. PSUM->SBUF output staging copies run on the ACT engine (DVE was the
busier engine). All PE operands are fp16; PSUM stays fp32. Per-core PE
streaming ~54k nominal cycles vs ~79k for the f32r/mask-matmul/per-tile-sums
baseline - and fp16 moving operands stream ~2 cols/cycle on HW, which f32r
does not, so the PE-bound steady state roughly halves on top of that.

Each core emits unnormalized PV partials (pvt [128 d, 2048 q perm]) and
denominators (sums [1, 2048]); the host un-permutes and combines the pair:
  out[b] = ((pv0 + pv1) / (s0 + s1)).T

Measured (reps-delta wall-clock, see prof.py): rel err 5.4e-4; ~12-13
us/iter steady state in the device's fast phase (best rounds 9.3-11.4 us;
the shared device stretches ~25% in slow phases) vs ~30.6 us for the staged
baseline under the same methodology (grader-reported baseline: 26.6 us).
Compute-bound: a probe build that skips the 4 MB xt upload still takes
~12.4 us/iter, i.e. PE streaming is ~90% of the critical path.
Phase-controlled A/Bs settled the emission structure: sequential per-chunk
emission beats software-pipelining (attention one group behind projection);
one DMA stream per hwdge ring beats splitting xt across SP+ACT; in-block
tile order [first, masked, ..., plain-last] is kept. The pipeline= and
split_rings= flags on _build_module reproduce the losing variants.
"""

import os

os.environ.setdefault("MYCRO_LOCAL_CACHE", "1")

import numpy as np

B, S, E, D = 4, 2048, 1024, 128
P = 128
NT = S // P          # 16 global s-tiles per batch
LT = NT // 2         # 8 local (per-core) k-tiles
NQB = 4              # 512-wide query blocks
QBW = 512
NEB = E // P         # 8 e-tiles
SCALE = 1.0 / float(np.sqrt(D))
NEG = -60000.0       # fp16-representable; exp underflows to 0 in fp32

TRACE = False        # set by test.py for profiling runs
TRACE_KW = {}
PROBE_NO_XT = False  # timing probe: skip xt DMA (results wrong; perf only)

_CACHE = {}


def _build_module(reps=1, pipeline=False, split_rings=False, fp8qk=False):
    from contextlib import ExitStack

    import concourse.bacc as bacc
    import concourse.mybir as mybir
    import concourse.tile as tile

    f32 = mybir.dt.float32
    f16 = mybir.dt.float16
    f8 = mybir.dt.float8e4

    nc = bacc.Bacc("TRN2", target_bir_lowering=False, debug=False, num_devices=8)

    if fp8qk:
        # e = 256*p4 + 2*ki + ko interleave for DoubleRow (K=256 per pass)
        x8_d = nc.dram_tensor("x8", [P, 4, 2, S], f8, kind="ExternalInput").ap()
        xtv_d = nc.dram_tensor("xtv", [E, S // 2], f16, kind="ExternalInput").ap()
        wq8_d = nc.dram_tensor("wq8", [P, 4, 2, D], f8, kind="ExternalInput").ap()
        wk8_d = nc.dram_tensor("wk8", [P, 4, 2, D], f8, kind="ExternalInput").ap()
    else:
        xt_d = nc.dram_tensor("xt", [E, S], f16, kind="ExternalInput").ap()
    wq_d = nc.dram_tensor("wq", [E, D], f16, kind="ExternalInput").ap()
    wk_d = nc.dram_tensor("wk", [E, D], f16, kind="ExternalInput").ap()
    wv_d = nc.dram_tensor("wv", [E, D], f16, kind="ExternalInput").ap()
    bq_d = nc.dram_tensor("bq", [D], f32, kind="ExternalInput").ap()  # pre-scaled
    bk_d = nc.dram_tensor("bk", [D], f32, kind="ExternalInput").ap()
    bv_d = nc.dram_tensor("bv", [D], f32, kind="ExternalInput").ap()
    mask_d = nc.dram_tensor("mask", [P, 2 * P], f16, kind="ExternalInput").ap()
    ones_d = nc.dram_tensor("ones", [P, 1], f16, kind="ExternalInput").ap()
    ident_d = nc.dram_tensor("ident", [P, P], f16, kind="ExternalInput").ap()
    pvt_d = nc.dram_tensor("pvt", [D, S], f32, kind="ExternalOutput").ap()
    sums_d = nc.dram_tensor("sums", [1, S], f32, kind="ExternalOutput").ap()

    with tile.TileContext(nc) as tc, ExitStack() as ctx:
        singles = ctx.enter_context(tc.tile_pool(name="singles", bufs=1))
        ppool = ctx.enter_context(tc.tile_pool(name="ppool", bufs=4))
        apool = ctx.enter_context(tc.tile_pool(name="apool", bufs=2))
        proj_ps = ctx.enter_context(tc.tile_pool(name="proj_ps", bufs=1, space="PSUM"))
        tr_ps = proj_ps
        sc_ps = ctx.enter_context(tc.tile_pool(name="sc_ps", bufs=3, space="PSUM"))
        pv_ps = ctx.enter_context(tc.tile_pool(name="pv_ps", bufs=1, space="PSUM"))
        sum_ps = ctx.enter_context(tc.tile_pool(name="sum_ps", bufs=1, space="PSUM"))

        # ---- constants (ACT HWDGE ring; xt stream owns the SP ring) ----
        w_sb = {}
        w_names = (("wv", wv_d),) if fp8qk else (
            ("wk", wk_d), ("wv", wv_d), ("wq", wq_d))
        for name, dram in w_names:
            t = singles.tile([P, NEB, D], f16, tag=f"w_{name}")
            nc.scalar.dma_start(t[:], dram.rearrange("(o p) d -> p o d", p=P))
            w_sb[name] = t
        w8 = {}
        if fp8qk:
            for name, dram in (("wq8", wq8_d), ("wk8", wk8_d)):
                t = singles.tile([P, 4, 2, D], f8, tag=f"w_{name}")
                nc.scalar.dma_start(t[:], dram[:])
                w8[name] = t
        b_sb = {}
        for name, dram in (("bq", bq_d), ("bk", bk_d), ("bv", bv_d)):
            t = singles.tile([P, 1], f32, tag=f"b_{name}")
            nc.scalar.dma_start(t[:], dram.rearrange("(p one) -> p one", one=1))
            b_sb[name] = t
        mask_sb = singles.tile([P, 2 * P], f16, tag="mask")
        nc.scalar.dma_start(mask_sb[:], mask_d[:])
        ones = singles.tile([P, 1], f16, tag="ones")
        nc.scalar.dma_start(ones[:], ones_d[:])
        ident = singles.tile([P, P], f16, tag="ident")
        nc.scalar.dma_start(ident[:], ident_d[:])

        # ---- persistent activations ----
        # xt resident in perm order: [e-part, e-tile, pair, parity-pos, col]
        if fp8qk:
            x8_sb = singles.tile([P, 4, 2, LT, 2, P], f8, tag="x8_sb")
            xtv_sb = singles.tile([P, NEB, LT, P], f16, tag="xtv_sb")
        else:
            xt_sb = singles.tile([P, NEB, LT, 2, P], f16, tag="xt_sb")
        kt = singles.tile([P, LT, P], f16, tag="kt")      # K^T  [d, lt, k]
        vt = singles.tile([P, LT, P], f16, tag="vt")      # V^T  [d, lt, k]
        vn = singles.tile([P, LT, D], f16, tag="vn")      # V natural [k, lt, d]
        qt = singles.tile([P, NT, P], f16, tag="qt")      # Q^T [d, pos, q]
        pvt_sb = singles.tile([D, S], f32, tag="pvt_sb")
        sums_sb = singles.tile([1, S], f32, tag="sums_sb")

        ktv = kt.rearrange("p lt k -> p (lt k)")
        vtv = vt.rearrange("p lt k -> p (lt k)")

        def load_chunk(J, rep=0):
            """DMA xt columns [512J, 512J+512) (positions 4J..4J+3)."""
            if PROBE_NO_XT and rep > 0:
                return
            if fp8qk:
                nc.sync.dma_start(
                    x8_sb[:, :, :, 2 * J : 2 * J + 2, :, :],
                    x8_d[:, :, :, J * QBW : (J + 1) * QBW],
                )
                for eo in range(NEB):
                    nc.sync.dma_start(
                        xtv_sb[:, eo, 2 * J : 2 * J + 2, :],
                        xtv_d[eo * P : (eo + 1) * P, J * 2 * P : (J + 1) * 2 * P],
                    )
                return
            for eo in range(NEB):
                eng = nc.sync if (eo % 2 == 0 or not split_rings) else nc.scalar
                eng.dma_start(
                    xt_sb[:, eo, 2 * J : 2 * J + 2, :, :],
                    xt_d[eo * P : (eo + 1) * P, J * QBW : (J + 1) * QBW],
                )

        def proj_chunk(J):
            """K/V for local k-tiles {2J, 2J+1} + Q for block J."""
            if fp8qk:
                # K: DoubleRow over full 512 perm cols (odd positions wasted,
                # but full-width keeps the rhs AP contiguous); keep evens.
                ps = proj_ps.tile([P, 2, 2, P], f32, tag="ps_kv")
                for p4 in range(4):
                    nc.tensor.matmul(
                        ps[:],
                        w8["wk8"][:, p4, :, :],
                        x8_sb[:, p4, :, 2 * J : 2 * J + 2, :, :],
                        start=(p4 == 0),
                        stop=(p4 == 3),
                        perf_mode=mybir.MatmulPerfMode.DoubleRow,
                    )
                nc.vector.tensor_scalar_add(
                    ktv[:, J * 2 * P : (J + 1) * 2 * P], ps[:, :, 0, :],
                    b_sb["bk"][:],
                )
                psv = proj_ps.tile([P, QBW], f32, tag="ps_kv")
                for eo in range(NEB):
                    nc.tensor.matmul(
                        psv[:, : 2 * P],
                        w_sb["wv"][:, eo, :],
                        xtv_sb[:, eo, 2 * J : 2 * J + 2, :],
                        start=(eo == 0),
                        stop=(eo == NEB - 1),
                    )
                nc.vector.tensor_scalar_add(
                    vtv[:, J * 2 * P : (J + 1) * 2 * P], psv[:, : 2 * P],
                    b_sb["bv"][:],
                )
                ps = proj_ps.tile([P, QBW], f32, tag="ps_q")
                for p4 in range(4):
                    nc.tensor.matmul(
                        ps[:],
                        w8["wq8"][:, p4, :, :],
                        x8_sb[:, p4, :, 2 * J : 2 * J + 2, :, :],
                        start=(p4 == 0),
                        stop=(p4 == 3),
                        perf_mode=mybir.MatmulPerfMode.DoubleRow,
                    )
                qv = qt.rearrange("p t q -> p (t q)")
                nc.vector.tensor_scalar(
                    qv[:, J * QBW : (J + 1) * QBW],
                    ps[:],
                    SCALE,
                    b_sb["bq"][:],
                    mybir.AluOpType.mult,
                    mybir.AluOpType.add,
                )
                return
            for name, dstv, bias in (("wk", ktv, "bk"), ("wv", vtv, "bv")):
                ps = proj_ps.tile([P, QBW], f32, tag="ps_kv")
                for eo in range(NEB):
                    nc.tensor.matmul(
                        ps[:, : 2 * P],
                        w_sb[name][:, eo, :],
                        xt_sb[:, eo, 2 * J : 2 * J + 2, 0, :],
                        start=(eo == 0),
                        stop=(eo == NEB - 1),
                    )
                nc.vector.tensor_scalar_add(
                    dstv[:, J * 2 * P : (J + 1) * 2 * P], ps[:, : 2 * P], b_sb[bias][:]
                )
            ps = proj_ps.tile([P, QBW], f32, tag="ps_q")
            for eo in range(NEB):
                nc.tensor.matmul(
                    ps[:],
                    w_sb["wq"][:, eo, :],
                    xt_sb[:, eo, 2 * J : 2 * J + 2, :, :],
                    start=(eo == 0),
                    stop=(eo == NEB - 1),
                )
            qv = qt.rearrange("p t q -> p (t q)")
            nc.vector.tensor_scalar(
                qv[:, J * QBW : (J + 1) * QBW],
                ps[:],
                SCALE,
                b_sb["bq"][:],
                mybir.AluOpType.mult,
                mybir.AluOpType.add,
            )

        def v_transpose(lt):
            ps = tr_ps.tile([P, P], f16, tag="tr")
            nc.tensor.transpose(ps[:], vt[:, lt, :], ident[:])
            nc.vector.tensor_copy(out=vn[:, lt, :], in_=ps[:])

        def attention_blk(J):
            """Block J: q-cols [512J, 512J+512), local k-tiles 0..2J+1."""
            nlt = 2 * J + 2
            col0 = J * QBW
            pv = pv_ps.tile([P, QBW], f32, tag="pv")
            acc = apool.tile([P, QBW], f16, tag="acc")
            # emission order: full-width i=0 first (clears PSUM over the
            # whole block), then the masked tiles (their DVE-mask + exp
            # latency hides behind the remaining scores matmuls), ending on
            # a plain tile so only one exp latency is exposed at block end.
            if J == 0:
                order = [0, 1]
            else:
                order = [0, 2 * J, 2 * J + 1] + list(range(1, 2 * J))
            for idx, i in enumerate(order):
                c0 = 2 * P if i == 2 * J + 1 else 0
                sc = sc_ps.tile([P, QBW], f32, tag="sc")
                nc.tensor.matmul(
                    sc[:, c0:],
                    kt[:, i, :],
                    qt[:, 4 * J + c0 // P : 4 * J + 4, :],
                    start=True,
                    stop=True,
                )
                if i >= 2 * J:
                    nc.vector.tensor_tensor(
                        out=sc[:, c0 : c0 + 2 * P],
                        in0=sc[:, c0 : c0 + 2 * P],
                        in1=mask_sb[:],
                        op=mybir.AluOpType.add,
                    )
                if idx == 0:
                    psrc = acc
                    nc.scalar.activation(
                        acc[:], sc[:], mybir.ActivationFunctionType.Exp
                    )
                else:
                    psrc = ppool.tile([P, QBW], f16, tag="p")
                    nc.scalar.activation(
                        psrc[:, c0:], sc[:, c0:], mybir.ActivationFunctionType.Exp
                    )
                    nc.vector.tensor_tensor(
                        out=acc[:, c0:],
                        in0=acc[:, c0:],
                        in1=psrc[:, c0:],
                        op=mybir.AluOpType.add,
                    )
                nc.tensor.matmul(
                    pv[:, c0:],
                    vn[:, i, :],
                    psrc[:, c0:],
                    start=(idx == 0),
                    stop=(idx == nlt - 1),
                    skip_group_check=True,
                )
            return pv, acc

        def finish_blk(J, pv, acc):
            """Block J tail: denominator matmul, staging copies, output DMA.
            Emitted one chunk-group after attention_blk(J) so the PE has
            projection work queued while ACT/DVE drain the block tail."""
            col0 = J * QBW
            sm = sum_ps.tile([1, QBW], f32, tag="sm")
            nc.tensor.matmul(sm[:], ones[:], acc[:], start=True, stop=True)
            nc.scalar.copy(out=pvt_sb[:, col0 : col0 + QBW], in_=pv[:])
            nc.scalar.copy(out=sums_sb[:, col0 : col0 + QBW], in_=sm[:])
            out_eng = nc.gpsimd
            out_eng.dma_start(
                pvt_d[:, col0 : col0 + QBW], pvt_sb[:, col0 : col0 + QBW]
            )
            out_eng.dma_start(
                sums_d[:, col0 : col0 + QBW], sums_sb[:, col0 : col0 + QBW]
            )

        # ---- emission order (priority hint for the scheduler) ----
        # Software pipeline: group k emits chunk-k%4's DMA + projections,
        # then finish(k-2), then attention(k-1). Attention never sits at the
        # head of the PE queue behind its own block tail, and each block's
        # exposed exp/acc latency is covered by the next chunk's projections
        # (across the rep boundary too).
        pend_attn = None   # J of attention not yet emitted
        pend_fin = None    # (J, pv, acc) of finish not yet emitted
        for _rep in range(reps):
            for J in range(NQB):
                load_chunk(J, _rep)
                proj_chunk(J)
                v_transpose(2 * J)
                v_transpose(2 * J + 1)
                if not pipeline:
                    finish_blk(J, *attention_blk(J))
                    continue
                if pend_fin is not None:
                    finish_blk(*pend_fin)
                    pend_fin = None
                if pend_attn is not None:
                    pend_fin = (pend_attn, *attention_blk(pend_attn))
                pend_attn = J
        # drain the pipeline tail
        if pend_fin is not None:
            finish_blk(*pend_fin)
        if pend_attn is not None:
            finish_blk(pend_attn, *attention_blk(pend_attn))

    nc.compile()
    return nc


def _get_module(reps=1, pipeline=False, split_rings=False, fp8qk=False):
    key = ("nc", reps, pipeline, split_rings, fp8qk)
    if key not in _CACHE:
        _CACHE[key] = _build_module(reps, pipeline, split_rings, fp8qk)
    return _CACHE[key]


def _host_prep(x, Wq, bq, Wk, bk, Wv, bv, fp8qk=False):
    """Build the 8 per-core input maps plus per-core q-column permutations.
    fp8qk=True adds the interleaved fp8 tensors for the (rejected) DoubleRow
    projection variant -- see the fp8qk flag on _build_module."""
    x = np.asarray(x, dtype=np.float32)
    tri = np.where(
        np.arange(P)[None, :] >= np.arange(P)[:, None], 0.0, NEG
    ).astype(np.float16)
    in_maps = []
    perms = []
    for c in range(8):
        b, h = divmod(c, 2)
        xt3 = np.ascontiguousarray(x[b].T).reshape(E, NT, P)
        # perm order: position p holds global tile p^h
        pos = np.arange(NT) ^ h
        xt_perm = np.ascontiguousarray(xt3[:, pos, :].reshape(E, S)).astype(
            np.float16
        )
        mask = np.concatenate(
            [tri, np.full((P, P), 0.0 if h == 0 else NEG, np.float16)], axis=1
        )
        extra = {}
        if fp8qk:
            import ml_dtypes
            f8 = ml_dtypes.float8_e4m3
            x8 = np.ascontiguousarray(
                xt_perm.astype(np.float32).reshape(4, P, 2, S).transpose(1, 0, 2, 3)
            )
            w8q = np.asarray(Wq, np.float32).reshape(4, P, 2, D).transpose(1, 0, 2, 3)
            w8k = np.asarray(Wk, np.float32).reshape(4, P, 2, D).transpose(1, 0, 2, 3)
            extra = {
                "x8": x8.astype(f8),
                "xtv": np.ascontiguousarray(
                    xt_perm.reshape(E, NT, P)[:, 0::2, :].reshape(E, S // 2)
                ),
                "wq8": np.ascontiguousarray(w8q).astype(f8),
                "wk8": np.ascontiguousarray(w8k).astype(f8),
            }
        in_maps.append(
            {
                **extra,
                "xt": xt_perm,
                "wq": np.asarray(Wq, np.float16),
                "wk": np.asarray(Wk, np.float16),
                "wv": np.asarray(Wv, np.float16),
                "bq": np.asarray(bq, np.float32) * np.float32(SCALE),
                "bk": np.asarray(bk, np.float32),
                "bv": np.asarray(bv, np.float32),
                "mask": np.ascontiguousarray(mask),
                "ones": np.ones((P, 1), dtype=np.float16),
                "ident": np.eye(P, dtype=np.float16),
            }
        )
        # storage col -> global q row (position tile p holds global tile p^h)
        perm = np.empty(S, dtype=np.int64)
        for t in range(NT):
            perm[t * P : (t + 1) * P] = (t ^ h) * P + np.arange(P)
        perms.append(perm)
    return in_maps, perms


def kernel(x, Wq, bq, Wk, bk, Wv, bv):
    from concourse.bass_utils import run_bass_kernel_spmd

    nc = _get_module()
    in_maps, perms = _host_prep(x, Wq, bq, Wk, bk, Wv, bv)
    res = run_bass_kernel_spmd(
        nc,
        in_maps,
        core_ids=list(range(8)),
        trace=TRACE,
        **TRACE_KW,
    )
    _CACHE["last_result"] = res

    out = np.empty((B, S, D), dtype=np.float32)
    for b in range(B):
        r0, r1 = res.results[2 * b], res.results[2 * b + 1]
        pv = np.zeros((D, S), dtype=np.float64)
        sm = np.zeros((S,), dtype=np.float64)
        for r, perm in ((r0, perms[2 * b]), (r1, perms[2 * b + 1])):
            pv[:, perm] += r["pvt"].astype(np.float64)
            sm[perm] += r["sums"][0].astype(np.float64)
        out[b] = (pv / sm[None, :]).T.astype(np.float32)
    return out
